# revision 29
# baseline (speedup 1.0000x reference)
"""ClassAttention (decode-style single-query attention) on 8 TRN2 NeuronCores.

Math (per batch b):
    kv = x @ Wkv              # [N, 2*H*D], k half cols 0:1024, v half 1024:2048
    q  = x[0] @ Wq            # [H*D]  (CLS token only)
    logits[t, h] = scale * sum_d q[h,d] * k[t, h*64+d]
    attn = softmax_t(logits)
    cls[h,d] = sum_t attn[t,h] * v[t, h*64+d]
    out = cls @ Wproj + bproj

v3.3 structure:
  - All sweep matmuls are N=512 with TINY stationaries (LDWEIGHTS is 16
    cols and overlaps the previous matmul):
      logits^T[h, t] : lhsT = wkf block [128, 16], moving = x^T [128, 512]
      r[h, c]        : lhsT = e block   [128, 16], moving = x   [128, 512]
  - x^T comes from the DMA XBAR (SBUF->SBUF transposing DMA, one call per
    1024-row chunk) on the sync queue.
  - Queue discipline (the v3.2 lesson: DMAs issued on the scalar queue are
    IN-ORDER with ACT compute, so a data-gated DMA there blocks every
    later exp):
      gpsimd/SWDGE : x chunk cast-loads only (f32->bf16, 32KB/partition
                     contiguous descriptors)
      sync         : Wk full f32, then the 8 chunk transposes, then stores
      scalar       : only early, ungated loads (CLS rows, bproj, Wq blocks,
                     Wv/Wproj staged f32 + ACT bf16 casts) - all complete
                     before the first exp is needed
  - Attention accumulation lives in PSUM chains (64 matmuls/batch);
    sum_t exp is deferred to batch end (DVE reduces over kept e tiles).
  - Finalize is split: weight-independent part at sweep end; Wv/Wproj part
    emitted after the next batch's second chunk so it overlaps that sweep.
  - Softmax runs without max-subtraction (logits are O(1)); the 1/sum(exp)
    normalization is applied to the tiny r[h, c] tensor.

Sharding: pure data-parallel over B: 16 batches / 8 cores = 2 per core.
Weights are replicated; each core returns its [2, 1024] output shard.
"""

import numpy as np

import concourse.bass as bass
import concourse.mybir as mybir
import concourse.tile as tile
from concourse import bacc
from concourse.bass_utils import run_bass_kernel_spmd
from concourse.masks import make_identity

F32 = mybir.dt.float32
BF16 = mybir.dt.bfloat16

B, SEQ, C = 16, 4096, 1024
H, D = 16, 64
SCALE = D ** -0.5  # 0.125
N_CORES = 8
BPC = B // N_CORES          # batches per core
CB = C // 128               # 8 contraction blocks
RPC = 1024                  # seq rows per chunk
NCH = SEQ // RPC            # 4 chunks per batch
SUB = RPC // 128            # 8 sub-tiles (of 128 rows) per chunk


def _build():
    nc = bacc.Bacc(
        "TRN2", target_bir_lowering=False, debug=False, num_devices=N_CORES
    )
    x_ap = nc.dram_tensor("x", [BPC, SEQ, C], F32, kind="ExternalInput").ap()
    wq_ap = nc.dram_tensor("Wq", [C, H * D], F32, kind="ExternalInput").ap()
    wkv_ap = nc.dram_tensor("Wkv", [C, 2 * H * D], F32, kind="ExternalInput").ap()
    wp_ap = nc.dram_tensor("Wproj", [H * D, C], F32, kind="ExternalInput").ap()
    bp_ap = nc.dram_tensor("bproj", [C], F32, kind="ExternalInput").ap()
    out_ap = nc.dram_tensor("out", [BPC, C], F32, kind="ExternalOutput").ap()

    with tile.TileContext(nc) as tc:
        _emit(nc, tc, x_ap, wq_ap, wkv_ap, wp_ap, bp_ap, out_ap)
    nc.compile()
    return nc


def _emit_prefix(nc, tc, consts, wstage, wq_ap, wkv_ap, wp_ap, bp_ap, x_ap,
                 id16_f, wkf_bf):
    """Everything that must precede the sweeps:

    - CLS rows -> q (both batches in 2-row chains) -> qb broadcast via
      selector matmuls -> wkf fold (fused multiply-reduce on the DVE)
    - Wv/Wproj staged f32 on the scalar queue, cast to bf16 on ACT
    All scalar-queue DMAs here are ungated so they drain before any exp.
    Emitted BEFORE the x chunk loads so no prefix consumer ever waits on
    an x-load completion count (shared DMA semaphore lanes)."""
    with (
        tc.tile_pool(name="wkpool", bufs=1) as wkpool,
        tc.tile_pool(name="wkst", bufs=2) as wkst,
        tc.tile_pool(name="pre", bufs=1) as pre,
        tc.tile_pool(name="wbst", bufs=2) as wbst,
        tc.tile_pool(name="fold", bufs=2) as fold_pool,
        tc.tile_pool(name="qps", bufs=2, space="PSUM") as qps,
        tc.tile_pool(name="qbps", bufs=1, space="PSUM") as qbps,
        tc.tile_pool(name="xtps", bufs=1, space="PSUM") as xtps,
    ):
        qb_sb = [
            pre.tile([128, C], BF16, tag=f"qb{b}", name=f"qb{b}")
            for b in range(BPC)
        ]
        # CLS rows, naturally laid out (single-descriptor loads), then
        # transposed on the PE into xclsT[c % 128, g*16 + b] (bf16).
        xcls_nat = pre.tile([16, C], F32)
        nc.vector.memset(xcls_nat[:], 0.0)
        for b in range(BPC):
            nc.scalar.dma_start(xcls_nat[b : b + 1, :], x_ap[b, 0:1, :])
        bproj_sb = consts.tile([1, C], F32)
        nc.scalar.dma_start(bproj_sb[:], bp_ap[:].unsqueeze(0))

        xclsT_ps = xtps.tile([128, 128], F32, tag="xT")
        for g in range(CB):
            nc.tensor.transpose(
                xclsT_ps[:, g * H : (g + 1) * H],
                xcls_nat[:, g * 128 : (g + 1) * 128],
                id16_f[:],
            )
        xclsT = pre.tile([128, 128], BF16)
        nc.vector.tensor_copy(xclsT[:], xclsT_ps[:])

        # Wk staged f32 on the sync queue, ACT-cast to a resident bf16 copy
        wk_bf = wkpool.tile([128, CB, 1024], BF16, tag="wk")
        for g in range(CB):
            wkt = wkst.tile([128, 1024], F32, tag="wkt")
            nc.sync.dma_start(wkt[:], wkv_ap[g * 128 : (g + 1) * 128, 0:1024])
            nc.scalar.copy(wk_bf[:, g, :], wkt[:])

        # selector rows: sel[b][k, m] = SCALE if k == b else 0  (k in 0..1)
        sel0 = pre.tile([BPC, 128], F32, tag="sel0", name="sel0")
        nc.vector.memset(sel0[:], 0.0)
        nc.vector.memset(sel0[0:1, :], SCALE)
        sel1 = pre.tile([BPC, 128], F32, tag="sel1", name="sel1")
        nc.vector.memset(sel1[:], SCALE)
        nc.vector.memset(sel1[0:1, :], 0.0)
        sel = [sel0, sel1]

        # q chains (both batches at once): Wq streamed bf16 through staging
        q_ps = [
            qps.tile([BPC, 512], F32, tag="q", name=f"qps{ch}")
            for ch in range(2)
        ]
        for g in range(CB):
            wst = wstage.tile([128, 1024], F32, tag="wst")
            nc.scalar.dma_start(wst[:], wq_ap[g * 128 : (g + 1) * 128, :])
            wst_bf = wbst.tile([128, 1024], BF16, tag="wb")
            nc.scalar.copy(wst_bf[:], wst[:])
            for ch in range(2):
                nc.tensor.matmul(
                    q_ps[ch][:],
                    xclsT[:, g * H : g * H + BPC],
                    wst_bf[:, ch * 512 : (ch + 1) * 512],
                    start=(g == 0),
                    stop=(g == CB - 1),
                )
        q2_sb = pre.tile([BPC, C], F32)
        for ch in range(2):
            nc.vector.tensor_copy(
                q2_sb[:, ch * 512 : (ch + 1) * 512], q_ps[ch][:]
            )

        # qb[b][c_p, hd] = scale * q[b, hd] broadcast down partitions,
        # via the selector matmul (contraction over the 2 batch rows)
        for b in range(BPC):
            for ch in range(2):
                qb_ps = qbps.tile([128, 512], F32, tag="qb")
                nc.tensor.matmul(
                    qb_ps[:],
                    sel[b][:],
                    q2_sb[:, ch * 512 : (ch + 1) * 512],
                    start=True,
                    stop=True,
                )
                nc.vector.tensor_copy(
                    qb_sb[b][:, ch * 512 : (ch + 1) * 512], qb_ps[:]
                )

        # wkf[b][c, g, h] = scale * sum_d q[b,(h,d)] * Wk[c,(h,d)]; fused
        # multiply+reduce, batch 0 first so its logits start early
        for b in range(BPC):
            for g in range(CB):
                prod = fold_pool.tile([128, H * D], BF16, tag="prod")
                nc.vector.tensor_mul(prod[:], wk_bf[:, g, :], qb_sb[b][:])
                wkf_g = fold_pool.tile([128, H], F32, tag="wkfg")
                nc.vector.tensor_reduce(
                    wkf_g[:].unsqueeze(2),
                    prod[:].rearrange("p (h d) -> p h d", d=D),
                    axis=mybir.AxisListType.X,
                    op=mybir.AluOpType.add,
                )
                nc.vector.tensor_copy(wkf_bf[b][:, g, :], wkf_g[:])
    return bproj_sb


class _BatchState:
    def __init__(self):
        self.r_psA = None
        self.r_psB = None
        self.es = []        # 8 e_sb tiles per batch, kept for end-sums
        self.xts = {}
        self.pend = None
        self.r_sb = None


def _emit(nc, tc, x_ap, wq_ap, wkv_ap, wp_ap, bp_ap, out_ap):
    with tc.tile_pool(name="consts", bufs=1) as consts:
        wv_bf = consts.tile([128, CB, 1024], BF16)
        wp_bf = consts.tile([128, CB, 1024], BF16)

        id16_bf = consts.tile([16, 16], BF16)
        make_identity(nc, id16_bf[:])
        id16_f = consts.tile([16, 16], F32)
        make_identity(nc, id16_f[:])

        wkf_bf = [
            consts.tile([128, CB, H], BF16, tag=f"wkf{b}", name=f"wkf{b}")
            for b in range(BPC)
        ]
        with (
            tc.tile_pool(name="xbf", bufs=3) as xbf_pool,
            tc.tile_pool(name="xt", bufs=3) as xt_pool,
            tc.tile_pool(name="wstage", bufs=2) as wstage,
            tc.tile_pool(name="esb", bufs=10) as esb_pool,
            tc.tile_pool(name="ebf", bufs=4) as ebf_pool,
            tc.tile_pool(name="small", bufs=1) as small,
        ):
            # prefix first: its DMAs take early semaphore-lane counts so
            # no prefix consumer falsely waits on an x-chunk completion
            bproj_sb = _emit_prefix(
                nc, tc, consts, wstage, wq_ap, wkv_ap, wp_ap, bp_ap, x_ap,
                id16_f, wkf_bf,
            )

            # ---- SWDGE queue: x chunk cast-loads, emitted lazily so
            # other gpsimd-engine work can interleave in its FIFO ----
            x_tiles = {}
            chunk_order = [(b, k) for b in range(BPC) for k in range(NCH)]
            next_load = [0]

            def emit_xload():
                if next_load[0] >= len(chunk_order):
                    return
                b, k = chunk_order[next_load[0]]
                next_load[0] += 1
                x_bf = xbf_pool.tile([128, SUB, 1024], BF16, tag="x",
                                     name="x_bf")
                nc.gpsimd.dma_start(
                    x_bf[:],
                    x_ap[b, k * RPC : (k + 1) * RPC, :].rearrange(
                        "(p i) c -> p i c", p=128
                    ),
                )
                x_tiles[(b, k)] = x_bf

            for _ in range(3):
                emit_xload()

            sts = [_BatchState() for _ in range(BPC)]

            sweep_psum = tc.tile_pool(name="lgps", bufs=3, space="PSUM")
            lgps = sweep_psum.__enter__()
            xat_psum = tc.tile_pool(name="xatps", bufs=2, space="PSUM")
            xatps = xat_psum.__enter__()
            t16_psum = tc.tile_pool(name="t16ps", bufs=2, space="PSUM")
            t16ps = t16_psum.__enter__()

            def emit_xt(b, k):
                if (k, 0) in sts[b].xts:
                    return
                for h in range(2):
                    xt = xt_pool.tile([128, 4 * CB, 128], BF16, tag="xt")
                    nc.sync.dma_start(
                        xt[:], x_tiles[(b, k)][:, 4 * h : 4 * h + 4, :],
                        transpose=True,
                    )
                    sts[b].xts[(k, h)] = xt

            def emit_logits(b, k):
                st = sts[b]
                es = []
                for grp in range(2):
                    xt = st.xts[(k, grp)]
                    lg = lgps.tile([16, 512], F32, tag="lg")
                    for g in range(CB):
                        nc.tensor.matmul(
                            lg[:],
                            wkf_bf[b][:, g, :],
                            xt[:, g : g + 3 * CB + 1 : CB, :],
                            start=(g == 0),
                            stop=(g == CB - 1),
                        )
                    e_sb = esb_pool.tile([16, 512], BF16, tag="e")
                    nc.scalar.activation(
                        e_sb[:], lg[:], mybir.ActivationFunctionType.Exp
                    )
                    es.append(e_sb)
                st.es.extend(es)
                st.pend = (k, es)

            def emit_racc(b):
                st = sts[b]
                if st.pend is None:
                    return
                k, es = st.pend
                st.pend = None
                x_bf = x_tiles[(b, k)]
                if st.r_psA is None:
                    st.r_psA = xatps.tile([16, 512], F32, tag="xat", name="rA")
                    st.r_psB = xatps.tile([16, 512], F32, tag="xat", name="rB")
                # all 8 eT transposes into one PSUM tile, one DVE copy out
                eT = t16ps.tile([128, SUB * H], BF16, tag="e16")
                for grp in range(2):
                    for j in range(4):
                        i = grp * 4 + j
                        nc.tensor.transpose(
                            eT[:, i * H : (i + 1) * H],
                            es[grp][:, j * 128 : (j + 1) * 128],
                            id16_bf[:],
                        )
                e_bf = ebf_pool.tile([128, SUB * H], BF16, tag="ebf")
                nc.vector.tensor_copy(e_bf[:], eT[:])
                for i in range(SUB):
                    first = k == 0 and i == 0
                    last = k == NCH - 1 and i == SUB - 1
                    nc.tensor.matmul(
                        st.r_psA[:], e_bf[:, i * H : (i + 1) * H],
                        x_bf[:, i, 0:512], start=first, stop=last,
                    )
                    nc.tensor.matmul(
                        st.r_psB[:], e_bf[:, i * H : (i + 1) * H],
                        x_bf[:, i, 512:1024], start=first, stop=last,
                    )

            def sweep_chunks(b, ks):
                """Depth-1 software pipeline: racc(k-1) is emitted after
                logits(k), so the PE never waits on the ACT exp."""
                st = sts[b]
                for k in ks:
                    if k == 0:
                        emit_xt(b, 0)
                    prev = st.pend
                    emit_logits(b, k)   # sets st.pend = (k, es)
                    cur = st.pend
                    st.pend = prev
                    emit_xload()
                    nxt = (b, k + 1) if k + 1 < NCH else (b + 1, 0)
                    if nxt in x_tiles:
                        emit_xt(*nxt)
                    emit_racc(b)        # racc for chunk k-1 (if any)
                    st.pend = cur

            def fin_part1(b):
                """Weight-independent: scale r by 1/sum(exp); frees PSUM."""
                st = sts[b]
                sums_all = small.tile([16, CB], F32, tag="sall", name="sall")
                for gi, e_sb in enumerate(st.es):
                    nc.vector.tensor_reduce(
                        sums_all[:, gi : gi + 1], e_sb[:],
                        axis=mybir.AxisListType.X, op=mybir.AluOpType.add,
                    )
                sums = small.tile([16, 1], F32, tag="sums", name="sums")
                nc.vector.tensor_reduce(
                    sums[:], sums_all[:],
                    axis=mybir.AxisListType.X, op=mybir.AluOpType.add,
                )
                rec = small.tile([16, 1], F32, tag="rec", name="rec")
                nc.vector.reciprocal(rec[:], sums[:])
                r_sb = small.tile([16, C], F32, tag="rsb", name="rsb")
                nc.vector.tensor_scalar_mul(r_sb[:, 0:512], st.r_psA[:], rec[:])
                nc.vector.tensor_scalar_mul(r_sb[:, 512:1024], st.r_psB[:],
                                            rec[:])
                st.r_sb = r_sb

            def fin_part2(b):
                """Needs wv_bf / wp_bf."""
                st = sts[b]
                r_bf = small.tile([16, C], BF16, tag="rbf", name="rbf")
                nc.vector.tensor_copy(r_bf[:], st.r_sb[:])
                rT_ps = t16ps.tile([128, CB * H], BF16, tag="e16")
                for g in range(CB):
                    nc.tensor.transpose(
                        rT_ps[:, g * H : (g + 1) * H],
                        r_bf[:, g * 128 : (g + 1) * 128],
                        id16_bf[:],
                    )
                rT_bf = small.tile([128, CB, H], BF16, tag="rT", name="rT")
                nc.vector.tensor_copy(
                    rT_bf[:].rearrange("p g h -> p (g h)"), rT_ps[:]
                )

                cls_bf = small.tile([16, C], BF16, tag="cls", name="cls")
                for ch in range(2):
                    cls_ps = lgps.tile([16, 512], F32, tag="lg")
                    for g in range(CB):
                        nc.tensor.matmul(
                            cls_ps[:],
                            rT_bf[:, g, :],
                            wv_bf[:, g, ch * 512 : (ch + 1) * 512],
                            start=(g == 0),
                            stop=(g == CB - 1),
                        )
                    nc.vector.tensor_copy(
                        cls_bf[:, ch * 512 : (ch + 1) * 512], cls_ps[:]
                    )

                # diagonal pick: clsv[hd] = cls_bf[hd//64, hd]
                aT = t16ps.tile([128, CB * H], BF16, tag="e16")
                for g in range(CB):
                    nc.tensor.transpose(
                        aT[:, g * H : (g + 1) * H],
                        cls_bf[:, g * 128 : (g + 1) * 128],
                        id16_bf[:],
                    )
                clsv_bf = small.tile([128, CB], BF16, tag="cv", name="cv")
                for g in range(CB):
                    for half in range(2):
                        rows = slice(64 * half, 64 * half + 64)
                        col = g * H + 2 * g + half
                        nc.vector.tensor_copy(
                            clsv_bf[rows, g : g + 1], aT[rows, col : col + 1]
                        )

                o_sb = small.tile([1, C], F32, tag="osb", name="osb")
                for ch in range(2):
                    o_ps = lgps.tile([16, 512], F32, tag="lg")
                    for g in range(CB):
                        nc.tensor.matmul(
                            o_ps[0:1, :],
                            clsv_bf[:, g : g + 1],
                            wp_bf[:, g, ch * 512 : (ch + 1) * 512],
                            start=(g == 0),
                            stop=(g == CB - 1),
                        )
                    nc.vector.tensor_add(
                        o_sb[0:1, ch * 512 : (ch + 1) * 512],
                        o_ps[0:1, :],
                        bproj_sb[0:1, ch * 512 : (ch + 1) * 512],
                    )
                nc.sync.dma_start(out_ap[b : b + 1, :], o_sb[:])

            def emit_wstage(dst, src_ap, c0):
                """f32 stage on sync (issue-only cost there), bf16 cast on
                the otherwise-idle gpsimd engine."""
                for g in range(CB):
                    wst = wstage.tile([128, 1024], F32, tag="wst")
                    nc.sync.dma_start(
                        wst[:], src_ap[g * 128 : (g + 1) * 128, c0 : c0 + 1024]
                    )
                    nc.gpsimd.tensor_copy(dst[:, g, :], wst[:])

            # ---- global schedule ----
            sweep_chunks(0, [0, 1])
            emit_wstage(wv_bf, wkv_ap, 1024)
            sweep_chunks(0, [2, 3])
            emit_racc(0)                # drain chunk 3
            fin_part1(0)
            emit_wstage(wp_bf, wp_ap, 0)
            sweep_chunks(1, [0, 1])
            fin_part2(0)
            sweep_chunks(1, [2, 3])
            emit_racc(1)
            fin_part1(1)
            fin_part2(1)

            t16_psum.__exit__(None, None, None)
            xat_psum.__exit__(None, None, None)
            sweep_psum.__exit__(None, None, None)


_CACHED = None


def _get_program():
    global _CACHED
    if _CACHED is None:
        _CACHED = _build()
    return _CACHED


def kernel(x, Wq, Wkv, Wproj, bproj, _trace=False):
    x = np.ascontiguousarray(np.asarray(x, dtype=np.float32))
    Wq = np.ascontiguousarray(np.asarray(Wq, dtype=np.float32))
    Wkv = np.ascontiguousarray(np.asarray(Wkv, dtype=np.float32))
    Wproj = np.ascontiguousarray(np.asarray(Wproj, dtype=np.float32))
    bproj = np.ascontiguousarray(np.asarray(bproj, dtype=np.float32))

    nc = _get_program()
    in_maps = [
        {
            "x": x[cid * BPC : (cid + 1) * BPC],
            "Wq": Wq,
            "Wkv": Wkv,
            "Wproj": Wproj,
            "bproj": bproj,
        }
        for cid in range(N_CORES)
    ]
    res = run_bass_kernel_spmd(
        nc, in_maps, core_ids=list(range(N_CORES)), trace=_trace
    )
    out = np.concatenate([res.results[cid]["out"] for cid in range(N_CORES)], axis=0)
    if _trace:
        kernel.last_exec_time_ns = res.exec_time_ns
        kernel.last_results = res
    return out.reshape(B, 1, C)


# revision 32
# speedup vs baseline: 1.2996x; 1.2996x over previous
"""ClassAttention (decode-style single-query attention) on 8 TRN2 NeuronCores.

Math (per batch b):
    kv = x @ Wkv              # [N, 2*H*D], k half cols 0:1024, v half 1024:2048
    q  = x[0] @ Wq            # [H*D]  (CLS token only)
    logits[t, h] = scale * sum_d q[h,d] * k[t, h*64+d]
    attn = softmax_t(logits)
    cls[h,d] = sum_t attn[t,h] * v[t, h*64+d]
    out = cls @ Wproj + bproj

v4 (hybrid of the proven v2 pipeline skeleton and the large-N math):
  - k / v are never materialized: logits fold into wkf and the attention
    output is reassociated through x.
  - Sweep matmuls are N=512 with tiny 16-col stationaries:
      logits^T[h, t] : lhsT = wkf block [128, 16], moving = x^T [128, 4, 128]
      r[h, c]        : lhsT = e block   [128, 16], moving = x   [128, 512]
    r and sum_t(exp) accumulate in PSUM chains (the sums chain rides the
    same stationary as the r matmuls: one extra N=1 matmul against ones).
  - x^T per 4-tile group: even groups via PE transposes (copies alternate
    DVE/ACT), odd groups via one XBAR transposing DMA on the sync queue.
  - Queue plan: SWDGE carries ONLY x (16 x 2MB contiguous-per-partition
    cast-loads). Wq/Wk stage f32 on the scalar queue and Wv/Wproj on the
    sync queue, all cast to resident bf16 on ACT early (the ACT queue is
    drained before the first exp; data-gated DMAs never sit on it).
  - Softmax runs without max-subtraction (logits are O(1)); the 1/sum(exp)
    normalization is applied to the tiny r[h, c] tensor at finalize.

Sharding: pure data-parallel over B: 16 batches / 8 cores = 2 per core.
Weights are replicated; each core returns its [2, 1024] output shard.
"""

import numpy as np

import concourse.bass as bass
import concourse.mybir as mybir
import concourse.tile as tile
from concourse import bacc
from concourse.bass_utils import run_bass_kernel_spmd
from concourse.masks import make_identity

F32 = mybir.dt.float32
BF16 = mybir.dt.bfloat16

B, SEQ, C = 16, 4096, 1024
H, D = 16, 64
SCALE = D ** -0.5  # 0.125
N_CORES = 8
BPC = B // N_CORES          # batches per core
CB = C // 128               # 8 contraction blocks
GRP = 4                     # t-tiles per group
NG = SEQ // (GRP * 128)     # 8 groups per batch


def _build():
    nc = bacc.Bacc(
        "TRN2", target_bir_lowering=False, debug=False, num_devices=N_CORES
    )
    x_ap = nc.dram_tensor("x", [BPC, SEQ, C], F32, kind="ExternalInput").ap()
    wq_ap = nc.dram_tensor("Wq", [C, H * D], F32, kind="ExternalInput").ap()
    wkv_ap = nc.dram_tensor("Wkv", [C, 2 * H * D], F32, kind="ExternalInput").ap()
    wp_ap = nc.dram_tensor("Wproj", [H * D, C], F32, kind="ExternalInput").ap()
    bp_ap = nc.dram_tensor("bproj", [C], F32, kind="ExternalInput").ap()
    out_ap = nc.dram_tensor("out", [BPC, C], F32, kind="ExternalOutput").ap()

    with tile.TileContext(nc) as tc:
        _emit(nc, tc, x_ap, wq_ap, wkv_ap, wp_ap, bp_ap, out_ap)
    nc.compile()
    return nc


def _emit(nc, tc, x_ap, wq_ap, wkv_ap, wp_ap, bp_ap, out_ap):
    with tc.tile_pool(name="consts", bufs=1) as consts:
        wv_bf = consts.tile([128, CB * 1024], BF16)
        wp_bf = consts.tile([128, CB * 1024], BF16)
        bproj_sb = consts.tile([1, C], F32)
        nc.sync.dma_start(bproj_sb[:], bp_ap[:].unsqueeze(0))

        id16_bf = consts.tile([16, 16], BF16)
        make_identity(nc, id16_bf[:])
        id16_f = consts.tile([16, 16], F32)
        make_identity(nc, id16_f[:])
        id128 = consts.tile([128, 128], BF16)
        make_identity(nc, id128[:])
        ones_bf = consts.tile([128, 1], BF16)
        nc.vector.memset(ones_bf[:], 1.0)

        wkf_bf = [
            consts.tile([128, CB * H], BF16, tag=f"wkf{b}", name=f"wkf{b}")
            for b in range(BPC)
        ]

        with (
            tc.tile_pool(name="xbf", bufs=5) as xbf_pool,
            tc.tile_pool(name="xt", bufs=3) as xt_pool,
            tc.tile_pool(name="wstage", bufs=3) as wstage,
            tc.tile_pool(name="esb", bufs=4) as esb_pool,
            tc.tile_pool(name="ebf", bufs=4) as ebf_pool,
            tc.tile_pool(name="small", bufs=1) as small,
        ):
            # ---------------- prefix ----------------
            with (
                tc.tile_pool(name="wqk", bufs=1) as wqk,
                tc.tile_pool(name="pre", bufs=1) as pre,
                tc.tile_pool(name="fold", bufs=2) as fold_pool,
                tc.tile_pool(name="qps", bufs=2, space="PSUM") as qps,
                tc.tile_pool(name="qbps", bufs=1, space="PSUM") as qbps,
                tc.tile_pool(name="xtps", bufs=1, space="PSUM") as xtps,
            ):
                # CLS rows (single-descriptor loads on sync), PE-transposed
                xcls_nat = pre.tile([16, C], F32)
                nc.vector.memset(xcls_nat[:], 0.0)
                for b in range(BPC):
                    nc.sync.dma_start(xcls_nat[b : b + 1, :], x_ap[b, 0:1, :])
                xclsT_ps = xtps.tile([128, 128], F32, tag="xT")
                for g in range(CB):
                    nc.tensor.transpose(
                        xclsT_ps[:, g * H : (g + 1) * H],
                        xcls_nat[:, g * 128 : (g + 1) * 128],
                        id16_f[:],
                    )
                xclsT = pre.tile([128, 128], BF16)
                nc.vector.tensor_copy(xclsT[:], xclsT_ps[:])

                wq_bf = wqk.tile([128, CB * 1024], BF16, tag="wq")
                wk_bf = wqk.tile([128, CB * 1024], BF16, tag="wk")
                for dst, src_ap, c0 in ((wq_bf, wq_ap, 0), (wk_bf, wkv_ap, 0)):
                    for g in range(CB):
                        wst = wstage.tile([128, 1024], F32, tag="wst")
                        nc.scalar.dma_start(
                            wst[:],
                            src_ap[g * 128 : (g + 1) * 128, c0 : c0 + 1024],
                        )
                        nc.scalar.copy(dst[:, g * 1024 : (g + 1) * 1024], wst[:])

                # selector rows: sel[b][k, m] = SCALE if k == b else 0
                sel0 = pre.tile([BPC, 128], F32, tag="sel0", name="sel0")
                nc.vector.memset(sel0[:], 0.0)
                nc.vector.memset(sel0[0:1, :], SCALE)
                sel1 = pre.tile([BPC, 128], F32, tag="sel1", name="sel1")
                nc.vector.memset(sel1[:], SCALE)
                nc.vector.memset(sel1[0:1, :], 0.0)
                sel = [sel0, sel1]

                # q for both batches at once: [2, 512] chains over c blocks
                q_ps = [
                    qps.tile([BPC, 512], F32, tag="q", name=f"qps{ch}")
                    for ch in range(2)
                ]
                for g in range(CB):
                    for ch in range(2):
                        nc.tensor.matmul(
                            q_ps[ch][:],
                            xclsT[:, g * H : g * H + BPC],
                            wq_bf[:, g * 1024 + ch * 512 : g * 1024 + (ch + 1) * 512],
                            start=(g == 0),
                            stop=(g == CB - 1),
                        )
                q2_sb = pre.tile([BPC, C], F32)
                for ch in range(2):
                    nc.vector.tensor_copy(
                        q2_sb[:, ch * 512 : (ch + 1) * 512], q_ps[ch][:]
                    )

                # qb[b] = scale * q[b] broadcast down partitions (bf16)
                qb_sb = [
                    pre.tile([128, C], BF16, tag=f"qb{b}", name=f"qb{b}")
                    for b in range(BPC)
                ]
                for b in range(BPC):
                    for ch in range(2):
                        qb_ps = qbps.tile([128, 512], F32, tag="qb")
                        nc.tensor.matmul(
                            qb_ps[:],
                            sel[b][:],
                            q2_sb[:, ch * 512 : (ch + 1) * 512],
                            start=True,
                            stop=True,
                        )
                        nc.vector.tensor_copy(
                            qb_sb[b][:, ch * 512 : (ch + 1) * 512], qb_ps[:]
                        )

                # wkf[b][c, (g h)] = scale * sum_d q[b,(h,d)] * Wk[c,(h,d)]
                for b in range(BPC):
                    for g in range(CB):
                        prod = fold_pool.tile([128, H * D], BF16, tag="prod")
                        nc.vector.tensor_mul(
                            prod[:], wk_bf[:, g * 1024 : (g + 1) * 1024],
                            qb_sb[b][:],
                        )
                        wkf_g = fold_pool.tile([128, H], F32, tag="wkfg")
                        nc.vector.tensor_reduce(
                            wkf_g[:].unsqueeze(2),
                            prod[:].rearrange("p (h d) -> p h d", d=D),
                            axis=mybir.AxisListType.X,
                            op=mybir.AluOpType.add,
                        )
                        nc.vector.tensor_copy(
                            wkf_bf[b][:, g * H : (g + 1) * H], wkf_g[:]
                        )

            # ---- SWDGE queue: x group cast-loads only ----
            x_tiles = {}
            for b in range(BPC):
                for tg in range(NG):
                    x_bf = xbf_pool.tile([128, GRP * C], BF16, tag="x",
                                         name="x_bf")
                    nc.gpsimd.dma_start(
                        x_bf[:].rearrange("p (i c) -> p i c", i=GRP),
                        x_ap[b, tg * GRP * 128 : (tg + 1) * GRP * 128, :]
                        .rearrange("(i t) c -> t i c", i=GRP),
                    )
                    x_tiles[(b, tg)] = x_bf

            # ---- Wv / Wproj staged f32 on sync, ACT bf16 casts ----
            for dst, src_ap, c0 in ((wv_bf, wkv_ap, 1024), (wp_bf, wp_ap, 0)):
                for g in range(CB):
                    wst = wstage.tile([128, 1024], F32, tag="wst")
                    nc.sync.dma_start(
                        wst[:], src_ap[g * 128 : (g + 1) * 128, c0 : c0 + 1024]
                    )
                    nc.scalar.copy(dst[:, g * 1024 : (g + 1) * 1024], wst[:])

            # ---------------- sweeps ----------------
            with (
                tc.tile_pool(name="trps", bufs=2, space="PSUM") as trps,
                tc.tile_pool(name="lgps", bufs=2, space="PSUM") as lgps,
                tc.tile_pool(name="raps", bufs=2, space="PSUM") as raps,
                tc.tile_pool(name="smps", bufs=1, space="PSUM") as smps,
                tc.tile_pool(name="e16ps", bufs=1, space="PSUM") as e16ps,
            ):
                state = {}

                def emit_xt(b, tg):
                    """x^T for group tg: [128, (i g) 128] with m = i*CB + g.
                    Even groups on the PE, odd groups via the XBAR DMA."""
                    x_bf = x_tiles[(b, tg)]
                    xt = xt_pool.tile([128, GRP * CB * 128], BF16, tag="xt",
                                      name="xt")
                    if tg % 2 == 1:
                        nc.sync.dma_start(
                            xt[:].rearrange("p (m c) -> p m c", c=128),
                            x_bf[:], transpose=True,
                        )
                    else:
                        for k in range(GRP * CB // 4):
                            tps = trps.tile([128, 512], BF16, tag="tps")
                            for j in range(4):
                                blk = k * 4 + j
                                nc.tensor.transpose(
                                    tps[:, j * 128 : (j + 1) * 128],
                                    x_bf[:, blk * 128 : (blk + 1) * 128],
                                    id128[:],
                                )
                            if k % 2 == 0:
                                nc.vector.tensor_copy(
                                    xt[:, k * 512 : (k + 1) * 512], tps[:]
                                )
                            else:
                                nc.scalar.copy(
                                    xt[:, k * 512 : (k + 1) * 512], tps[:]
                                )
                    return xt

                def emit_logits(b, tg, xt):
                    wkf = wkf_bf[b]
                    lg = lgps.tile([16, 512], F32, tag="lg")
                    xt_v = xt[:].rearrange("p (i g c) -> p i g c", i=GRP, g=CB)
                    for g in range(CB):
                        nc.tensor.matmul(
                            lg[:],
                            wkf[:, g * H : (g + 1) * H],
                            xt_v[:, :, g, :],
                            start=(g == 0),
                            stop=(g == CB - 1),
                        )
                    e_sb = esb_pool.tile([16, 512], BF16, tag="e")
                    nc.scalar.activation(
                        e_sb[:], lg[:], mybir.ActivationFunctionType.Exp
                    )
                    return e_sb

                def emit_racc(b, tg, e_sb):
                    st = state[b]
                    x_bf = x_tiles[(b, tg)]
                    eT = e16ps.tile([128, CB * H], BF16, tag="e16")
                    for j in range(GRP):
                        nc.tensor.transpose(
                            eT[:, j * H : (j + 1) * H],
                            e_sb[:, j * 128 : (j + 1) * 128],
                            id16_bf[:],
                        )
                    e_bf = ebf_pool.tile([128, GRP * H], BF16, tag="ebf")
                    nc.vector.tensor_copy(e_bf[:], eT[:, 0 : GRP * H])
                    for j in range(GRP):
                        first = tg == 0 and j == 0
                        last = tg == NG - 1 and j == GRP - 1
                        lhsT = e_bf[:, j * H : (j + 1) * H]
                        nc.tensor.matmul(
                            st["rA"][:], lhsT, x_bf[:, j * C : j * C + 512],
                            start=first, stop=last,
                        )
                        nc.tensor.matmul(
                            st["rB"][:], lhsT, x_bf[:, j * C + 512 : (j + 1) * C],
                            start=first, stop=last,
                        )
                        nc.tensor.matmul(
                            st["sm"][0:16, 0:1], lhsT, ones_bf[:],
                            start=first, stop=last,
                        )

                def sweep(b):
                    st = state.setdefault(b, {})
                    st["rA"] = raps.tile([16, 512], F32, tag="ra", name="rA")
                    st["rB"] = raps.tile([16, 512], F32, tag="ra", name="rB")
                    st["sm"] = smps.tile([16, 512], F32, tag="sm", name="sm")
                    pend = None
                    xt = emit_xt(b, 0)
                    for tg in range(NG):
                        e_sb = emit_logits(b, tg, xt)
                        if tg + 1 < NG:
                            xt = emit_xt(b, tg + 1)
                        if pend is not None:
                            emit_racc(b, *pend)
                        pend = (tg, e_sb)
                    emit_racc(b, *pend)

                def finalize(b):
                    st = state[b]
                    sums = small.tile([16, 1], F32, tag="sums", name="sums")
                    nc.vector.tensor_copy(sums[:], st["sm"][0:16, 0:1])
                    rec = small.tile([16, 1], F32, tag="rec", name="rec")
                    nc.vector.reciprocal(rec[:], sums[:])
                    r_bf = small.tile([16, C], BF16, tag="rbf", name="rbf")
                    nc.vector.tensor_scalar_mul(
                        r_bf[:, 0:512], st["rA"][:], rec[:]
                    )
                    nc.vector.tensor_scalar_mul(
                        r_bf[:, 512:1024], st["rB"][:], rec[:]
                    )

                    rT_ps = e16ps.tile([128, CB * H], BF16, tag="e16")
                    for g in range(CB):
                        nc.tensor.transpose(
                            rT_ps[:, g * H : (g + 1) * H],
                            r_bf[:, g * 128 : (g + 1) * 128],
                            id16_bf[:],
                        )
                    rT_bf = small.tile([128, CB * H], BF16, tag="rTb",
                                       name="rTb")
                    nc.vector.tensor_copy(rT_bf[:], rT_ps[:])

                    cls_bf = small.tile([16, C], BF16, tag="cls", name="cls")
                    for ch in range(2):
                        cls_ps = lgps.tile([16, 512], F32, tag="lg")
                        for g in range(CB):
                            nc.tensor.matmul(
                                cls_ps[:],
                                rT_bf[:, g * H : (g + 1) * H],
                                wv_bf[:, g * 1024 + ch * 512 :
                                      g * 1024 + (ch + 1) * 512],
                                start=(g == 0),
                                stop=(g == CB - 1),
                            )
                        nc.vector.tensor_copy(
                            cls_bf[:, ch * 512 : (ch + 1) * 512], cls_ps[:]
                        )

                    # diagonal pick: clsv[hd] = cls_bf[hd//64, hd]
                    aT = e16ps.tile([128, CB * H], BF16, tag="e16")
                    for g in range(CB):
                        nc.tensor.transpose(
                            aT[:, g * H : (g + 1) * H],
                            cls_bf[:, g * 128 : (g + 1) * 128],
                            id16_bf[:],
                        )
                    clsv_bf = small.tile([128, CB], BF16, tag="cv", name="cv")
                    for g in range(CB):
                        for half in range(2):
                            rows = slice(64 * half, 64 * half + 64)
                            col = g * H + 2 * g + half
                            nc.vector.tensor_copy(
                                clsv_bf[rows, g : g + 1], aT[rows, col : col + 1]
                            )

                    o_sb = small.tile([1, C], F32, tag="osb", name="osb")
                    for ch in range(2):
                        o_ps = lgps.tile([16, 512], F32, tag="lg")
                        for g in range(CB):
                            nc.tensor.matmul(
                                o_ps[0:1, :],
                                clsv_bf[:, g : g + 1],
                                wp_bf[:, g * 1024 + ch * 512 :
                                      g * 1024 + (ch + 1) * 512],
                                start=(g == 0),
                                stop=(g == CB - 1),
                            )
                        nc.vector.tensor_add(
                            o_sb[0:1, ch * 512 : (ch + 1) * 512],
                            o_ps[0:1, :],
                            bproj_sb[0:1, ch * 512 : (ch + 1) * 512],
                        )
                    nc.sync.dma_start(out_ap[b : b + 1, :], o_sb[:])

                sweep(0)
                finalize(0)
                sweep(1)
                finalize(1)


_CACHED = None


def _get_program():
    global _CACHED
    if _CACHED is None:
        _CACHED = _build()
    return _CACHED


def kernel(x, Wq, Wkv, Wproj, bproj, _trace=False):
    x = np.ascontiguousarray(np.asarray(x, dtype=np.float32))
    Wq = np.ascontiguousarray(np.asarray(Wq, dtype=np.float32))
    Wkv = np.ascontiguousarray(np.asarray(Wkv, dtype=np.float32))
    Wproj = np.ascontiguousarray(np.asarray(Wproj, dtype=np.float32))
    bproj = np.ascontiguousarray(np.asarray(bproj, dtype=np.float32))

    nc = _get_program()
    in_maps = [
        {
            "x": x[cid * BPC : (cid + 1) * BPC],
            "Wq": Wq,
            "Wkv": Wkv,
            "Wproj": Wproj,
            "bproj": bproj,
        }
        for cid in range(N_CORES)
    ]
    res = run_bass_kernel_spmd(
        nc, in_maps, core_ids=list(range(N_CORES)), trace=_trace
    )
    out = np.concatenate([res.results[cid]["out"] for cid in range(N_CORES)], axis=0)
    if _trace:
        kernel.last_exec_time_ns = res.exec_time_ns
        kernel.last_results = res
    return out.reshape(B, 1, C)
